# revision 28
# baseline (speedup 1.0000x reference)
"""DeepSeek-V3 MLA attention (B=1, S=1024, D=7168, H=128) on 8 Trainium2
NeuronCores.

Sharding: tensor-parallel over the 128 heads (16 heads/core).
Stage A (latent projections) is M-sharded (256 rows/core) and processed in
two token chunks; each chunk fires one AllReduce ([97,512] f32: 64 kpe rows
+ q/kv sum-of-squares partial rows at 64/96) and one combined q+kv
AllGather ([256,512] bf16) so chunk-0 collectives overlap chunk-1 compute
and downstream kv_b/q_b starts as soon as chunk 0 lands.  rsqrt factors are
broadcast across partitions with small indicator-matrix matmuls into PSUM
(no DRAM round trip).  Attention: all score matmuls are K=128 (rope keys
zero-padded via two kroped2 variants); softmax denominators accumulate in
PSUM via ones-matmuls per block; reciprocals are batched [4,T] per 4-head
group; head outputs are AllGathered (bf16) per group and o_proj is
row-sharded with an M-split (4+3) ch-inner loop so each weight tile feeds
two matmuls back to back.
"""

import os
from contextlib import ExitStack

import numpy as np
import ml_dtypes

import concourse.bass as bass
import concourse.mybir as mybir
import concourse.tile as tile
from concourse import bacc
from concourse.bass_utils import run_bass_kernel_spmd
from concourse.masks import make_upper_triangular

bf16 = ml_dtypes.bfloat16
F32 = mybir.dt.float32
BF = mybir.dt.bfloat16

B, S, D = 1, 1024, 7168
H, DN, DR, DV = 128, 128, 64, 128
DQ = DN + DR                  # 192
RQ, RKV = 1536, 512
EPS = 1e-6
SCALE = float(DQ) ** -0.5
NC = 8
HC = H // NC                  # 16 heads per core
T = S
TC = T // 2                   # token chunk
KT_X = D // 128               # 56
KT_Q = RQ // 128              # 12
KT_KV = RKV // 128            # 4
MT_QB = (HC * DQ) // 128      # 24 (16 nope tiles + 8 rope tiles)
DEBUG = bool(int(os.environ.get("BASSMLA_DEBUG", "0")))

_CACHE = {}


def _q_pieces(kt):
    """Global q-latent rows [128kt,128kt+128) -> (gathered_row, local_row,
    width) pieces of the [2048, TC] combined AllGather (per-core 256-row
    blocks: 192 q rows then 64 kv rows)."""
    segs = []
    r = 128 * kt
    end = r + 128
    while r < end:
        c = r // 192
        w = min(end, 192 * (c + 1)) - r
        segs.append((256 * c + (r - 192 * c), r - 128 * kt, w))
        r += w
    return segs


def _kv_pieces(kt):
    segs = []
    r = 128 * kt
    end = r + 128
    while r < end:
        c = r // 64
        w = min(end, 64 * (c + 1)) - r
        segs.append((256 * c + 192 + (r - 64 * c), r - 128 * kt, w))
        r += w
    return segs


def _build():
    nc = bacc.Bacc("TRN2", target_bir_lowering=False, debug=False, num_devices=NC)

    x_in = nc.dram_tensor("x", [128, KT_X, T], BF, kind="ExternalInput").ap()
    wa_in = nc.dram_tensor("wa", [128, KT_X, 2, 128], BF, kind="ExternalInput").ap()
    wkpe_in = nc.dram_tensor("wkpe", [128, 7, DR], BF, kind="ExternalInput").ap()
    ccss_in = nc.dram_tensor("ccss", [128, 2 * T], BF, kind="ExternalInput").ap()
    eall_in = nc.dram_tensor("eall", [4, 512], BF, kind="ExternalInput").ap()
    cst_in = nc.dram_tensor("cst", [2, 2], F32, kind="ExternalInput").ap()
    wqb_in = nc.dram_tensor("wqb", [128, MT_QB, KT_Q, 128], BF, kind="ExternalInput").ap()
    wkn_in = nc.dram_tensor("wkn", [128, HC, KT_KV, 128], BF, kind="ExternalInput").ap()
    wv_in = nc.dram_tensor("wv", [128, KT_KV, HC * DV], BF, kind="ExternalInput").ap()
    wo_in = nc.dram_tensor("wo", [128, H * DV // 128, 7, 128], BF, kind="ExternalInput").ap()
    out_ap = nc.dram_tensor("out", [D // NC, T], F32, kind="ExternalOutput").ap()
    if DEBUG:
        dbg_attn = nc.dram_tensor("dbg_attn", [HC * DV, T], F32, kind="ExternalOutput").ap()
        dbg_qn = nc.dram_tensor("dbg_qn", [RQ, T], F32, kind="ExternalOutput").ap()
        dbg_kv = nc.dram_tensor("dbg_kv", [RKV, T], F32, kind="ExternalOutput").ap()

    RG = [list(range(NC))]

    with tile.TileContext(nc) as tc:
        es_dram = ExitStack()
        dram = es_dram.enter_context(tc.tile_pool(name="dram", bufs=1, space="DRAM"))
        ar_in = [dram.tile([97, TC], F32, tag=f"ar_in{c}", name=f"ar_in{c}")
                 for c in range(2)]
        ar_out = [dram.tile([97, TC], F32, tag=f"ar_out{c}", name=f"ar_out{c}",
                            addr_space="Shared") for c in range(2)]
        ag_in = [dram.tile([256, TC], BF, tag=f"ag_in{c}", name=f"ag_in{c}")
                 for c in range(2)]
        ag_out = [dram.tile([256 * NC, TC], BF, tag=f"ag_out{c}",
                            name=f"ag_out{c}", addr_space="Shared")
                  for c in range(2)]
        ago_in = [dram.tile([4 * DV, T], BF, tag=f"ago_in{i}", name=f"ago_in{i}")
                  for i in range(4)]
        ago_out = [dram.tile([4 * DV * NC, T], BF, tag=f"ago_out{i}",
                             name=f"ago_out{i}", addr_space="Shared")
                   for i in range(4)]

        es_persist = ExitStack()
        persist = es_persist.enter_context(tc.tile_pool(name="persist", bufs=1))
        ones_bf = persist.tile([128, 1], BF, tag="ones", name="ones")
        nc.vector.memset(ones_bf, 1.0)
        # indicator matrices: e_all[k, 128i+m] = (k==i), for rank-k broadcasts
        e_all = persist.tile([4, 512], BF, tag="e_all", name="e_all")
        nc.sync.dma_start(out=e_all, in_=eall_in)
        # cst rows [q, kv]: col 0 = eps, col 1 = 1/R
        cst = persist.tile([2, 2], F32, tag="cst", name="cst")
        nc.sync.dma_start(out=cst, in_=cst_in)
        mask_t = persist.tile([128, 128], BF, tag="mask", name="mask")
        make_upper_triangular(nc, mask_t.opt(), val=1.0, diag=True)
        ccss_bf = persist.tile([128, 2 * T], BF, tag="ccss_bf", name="ccss_bf")
        nc.sync.dma_start(out=ccss_bf, in_=ccss_in)
        CCb = ccss_bf[:, 0:T]
        SSb = ccss_bf[:, T:2 * T]
        # rope-key tiles: variant a has kpe in partitions 0:64 (zeros below),
        # variant b has kpe in 64:128 (zeros above) -> all score MMs are K=128
        kroped2a = persist.tile([128, T], BF, tag="kr2a", name="kr2a")
        kroped2b = persist.tile([128, T], BF, tag="kr2b", name="kr2b")
        nc.vector.memset(kroped2a, 0.0)
        nc.vector.memset(kroped2b, 0.0)
        wkpe_t = persist.tile([128, 7, DR], BF, tag="wkpe", name="wkpe")
        nc.sync.dma_start(out=wkpe_t, in_=wkpe_in)

        # heads pool opens early (lives through attention); tiles alloc lazily
        es_heads = ExitStack()
        heads = es_heads.enter_context(tc.tile_pool(name="heads", bufs=1))

        # ============ Stage A: latent projections, 2 token chunks ============
        es_qnkv = ExitStack()
        qnkv = es_qnkv.enter_context(tc.tile_pool(name="qnkv", bufs=1))
        qn = [qnkv.tile([128, T], BF, tag=f"qn{k}", name=f"qn{k}") for k in range(KT_Q)]
        # v weights resident (freed with qnkv after q_b); wkn streams per tile
        wvr = qnkv.tile([128, KT_KV, HC * DV], BF, tag="wvr", name="wvr")
        for kt in range(KT_KV):
            nc.sync.dma_start(out=wvr[:, kt], in_=wv_in[:, kt])
        es_ckvp = ExitStack()
        ckvp = es_ckvp.enter_context(tc.tile_pool(name="ckvp", bufs=1))
        ckv = [ckvp.tile([128, T], BF, tag=f"ckv{k}", name=f"ckv{k}") for k in range(KT_KV)]

        es_xpool = ExitStack()
        xpool = es_xpool.enter_context(tc.tile_pool(name="xpool", bufs=4))
        es_sa = ExitStack()
        sa = es_sa.enter_context(tc.tile_pool(name="sa", bufs=2))
        es_sb = ExitStack()
        sb = es_sb.enter_context(tc.tile_pool(name="sb", bufs=1))
        es_psA = ExitStack()
        psA = es_psA.enter_context(tc.tile_pool(name="psA", bufs=1, space="PSUM"))
        es_psB = ExitStack()
        psB = es_psB.enter_context(tc.tile_pool(name="psB", bufs=1, space="PSUM"))

        for cch in range(2):
            cs = slice(TC * cch, TC * (cch + 1))
            psa = [psA.tile([128, TC], F32, tag=f"a{i}", name=f"a{i}")
                   for i in range(2)]
            # kko: rows 0:64 kpe partial, row 64 q sumsq, row 96 kv sumsq
            kko = psA.tile([97, TC], F32, tag="kko", name="kko")
            for kt in range(KT_X):
                xt = xpool.tile([128, TC], BF, tag="x", name="x")
                nc.sync.dma_start(out=xt, in_=x_in[:, kt, cs])
                wa_kt = xpool.tile([128, 2, 128], BF, tag="wa", name="wa")
                nc.sync.dma_start(out=wa_kt, in_=wa_in[:, kt])
                for mt in range(2):
                    nc.tensor.matmul(
                        psa[mt], wa_kt[:, mt, :], xt,
                        start=(kt == 0), stop=(kt == KT_X - 1))
                if kt < 7:
                    nc.tensor.matmul(
                        kko[0:64], wkpe_t[:, kt, :], xt,
                        start=(kt == 0), stop=(kt == 6), skip_group_check=True)
            t0_bf = sa.tile([128, TC], BF, tag="t0", name="t0")
            nc.scalar.copy(t0_bf, psa[0])
            t1_bf = sa.tile([128, TC], BF, tag="t1", name="t1")
            nc.scalar.copy(t1_bf, psa[1])
            # local sum-of-squares partials (q: 192 rows, kv: 64 rows)
            sq0 = sa.tile([128, TC], BF, tag="sq0", name="sq0")
            nc.vector.tensor_mul(sq0, t0_bf, t0_bf)
            sq1 = sa.tile([128, TC], BF, tag="sq1", name="sq1")
            nc.vector.tensor_mul(sq1, t1_bf, t1_bf)
            nc.tensor.matmul(kko[64:65], ones_bf, sq0,
                             start=True, stop=False, skip_group_check=True)
            nc.tensor.matmul(kko[64:65], ones_bf[0:64], sq1[0:64],
                             start=False, stop=True, skip_group_check=True)
            nc.tensor.matmul(kko[96:97], ones_bf[64:128], sq1[64:128],
                             start=True, stop=True, skip_group_check=True,
                             tile_position=(64, 96))
            ar_sb = sa.tile([97, TC], F32, tag="ar_sb", name="ar_sb")
            nc.scalar.copy(ar_sb, kko)
            nc.sync.dma_start(out=ar_in[cch][:], in_=ar_sb)
            nc.gpsimd.collective_compute(
                "AllReduce", mybir.AluOpType.add, replica_groups=RG,
                ins=[ar_in[cch].opt()], outs=[ar_out[cch].opt()])
            nc.sync.dma_start(out=ag_in[cch][0:128], in_=t0_bf)
            nc.sync.dma_start(out=ag_in[cch][128:256], in_=t1_bf)
            nc.gpsimd.collective_compute(
                "AllGather", mybir.AluOpType.bypass, replica_groups=RG,
                ins=[ag_in[cch].opt()], outs=[ag_out[cch].opt()])

            # -------- stage B for this chunk: consume AR + AG --------
            kpe_sb = sb.tile([64, TC], F32, tag="kpe_sb", name="kpe_sb")
            nc.sync.dma_start(out=kpe_sb, in_=ar_out[cch][0:64])
            kpe_sw = sb.tile([64, TC], F32, tag="kpe_sw", name="kpe_sw")
            nc.sync.dma_start(out=kpe_sw[0:32], in_=kpe_sb[32:64])
            nc.sync.dma_start(out=kpe_sw[32:64], in_=kpe_sb[0:32])
            kt1 = sb.tile([64, TC], F32, tag="kt1", name="kt1")
            kt2 = sb.tile([64, TC], F32, tag="kt2", name="kt2")
            nc.vector.tensor_mul(kt1, kpe_sb, CCb[0:64, cs])
            nc.vector.tensor_mul(kt2, kpe_sw, SSb[0:64, cs])
            kroped = sb.tile([64, TC], BF, tag="kroped", name="kroped")
            nc.vector.tensor_add(kroped, kt1, kt2)
            nc.sync.dma_start(out=kroped2a[0:64, cs], in_=kroped)
            nc.sync.dma_start(out=kroped2b[64:128, cs], in_=kroped)
            # inverse rms factors: gathered sumsq rows 64 (q) and 96 (kv)
            ss2 = sb.tile([2, TC], F32, tag="ss2", name="ss2")
            nc.sync.dma_start(out=ss2[0:1], in_=ar_out[cch][64:65])
            nc.sync.dma_start(out=ss2[1:2], in_=ar_out[cch][96:97])
            sroot = sb.tile([2, TC], F32, tag="sroot", name="sroot")
            nc.scalar.activation(out=sroot, in_=ss2,
                                 func=mybir.ActivationFunctionType.Sqrt,
                                 bias=cst[:, 0:1], scale=cst[:, 1:2])
            inv2 = sb.tile([2, TC], F32, tag="inv2", name="inv2")
            nc.vector.reciprocal(out=inv2, in_=sroot)
            inv2b = sb.tile([2, TC], BF, tag="inv2b", name="inv2b")
            nc.vector.tensor_copy(inv2b, inv2)
            bcq = psB.tile([128, TC], F32, tag="bcq", name="bcq")
            nc.tensor.matmul(bcq, e_all[0:2, 0:128], inv2b,
                             start=True, stop=True)
            bckv = psB.tile([128, TC], F32, tag="bckv", name="bckv")
            nc.tensor.matmul(bckv, e_all[0:2, 128:256], inv2b,
                             start=True, stop=True)
            # gather pieces -> qn/ckv tiles, then normalize in place
            for kt in range(KT_Q):
                for gs, lo, w in _q_pieces(kt):
                    nc.sync.dma_start(out=qn[kt][lo:lo + w, cs],
                                      in_=ag_out[cch][gs:gs + w])
                nc.vector.tensor_mul(qn[kt][:, cs], qn[kt][:, cs], bcq)
            for kt in range(KT_KV):
                for gs, lo, w in _kv_pieces(kt):
                    nc.sync.dma_start(out=ckv[kt][lo:lo + w, cs],
                                      in_=ag_out[cch][gs:gs + w])
                nc.vector.tensor_mul(ckv[kt][:, cs], ckv[kt][:, cs], bckv)

        es_psB.close()
        es_psA.close()
        es_sb.close()
        es_sa.close()
        es_xpool.close()

        if DEBUG:
            es_dq = ExitStack()
            dq = es_dq.enter_context(tc.tile_pool(name="dbgq", bufs=2))
            for k in range(KT_Q):
                t = dq.tile([128, T], F32, tag="d", name="d")
                nc.scalar.copy(t, qn[k])
                nc.sync.dma_start(out=dbg_qn[128 * k:128 * (k + 1)], in_=t)
            for k in range(KT_KV):
                t = dq.tile([128, T], F32, tag="d", name="d")
                nc.scalar.copy(t, ckv[k])
                nc.sync.dma_start(out=dbg_kv[128 * k:128 * (k + 1)], in_=t)
            es_dq.close()

        # ============ kv_b projections (cch = token half) ============
        kn = [heads.tile([128, T], BF, tag=f"kn{m}", name=f"kn{m}") for m in range(HC)]
        v_t = [heads.tile([128, HC * DV], BF, tag=f"v{t_}", name=f"v{t_}") for t_ in range(8)]

        es_s5 = ExitStack()
        s5 = es_s5.enter_context(tc.tile_pool(name="s5", bufs=2))
        es_ps5 = ExitStack()
        ps5 = es_ps5.enter_context(tc.tile_pool(name="ps5", bufs=3, space="PSUM"))
        for cch in range(2):
            cs = slice(TC * cch, TC * (cch + 1))
            for mt in range(HC):
                wknt = s5.tile([128, KT_KV, 128], BF, tag="wkn", name="wkn")
                nc.sync.dma_start(out=wknt, in_=wkn_in[:, mt])
                ps = ps5.tile([128, TC], F32, tag="ps", name="ps")
                for kt in range(KT_KV):
                    nc.tensor.matmul(ps, wknt[:, kt, :], ckv[kt][:, cs],
                                     start=(kt == 0), stop=(kt == KT_KV - 1))
                nc.scalar.copy(kn[mt][:, cs], ps)
            for tt in range(4 * cch, 4 * cch + 4):
                for c4 in range(4):
                    ps = ps5.tile([128, TC], F32, tag="ps", name="ps")
                    for kt in range(KT_KV):
                        nc.tensor.matmul(
                            ps, ckv[kt][:, 128 * tt:128 * (tt + 1)],
                            wvr[:, kt, 512 * c4:512 * (c4 + 1)],
                            start=(kt == 0), stop=(kt == KT_KV - 1))
                    nc.scalar.copy(v_t[tt][:, 512 * c4:512 * (c4 + 1)], ps)
        es_ps5.close()
        es_s5.close()
        es_ckvp.close()

        # ============ q_b projection + RoPE (cch = token half) ============
        qh = [heads.tile([128, T], BF, tag=f"qh{m}", name=f"qh{m}") for m in range(HC)]
        qr = [heads.tile([128, T], BF, tag=f"qr{m}", name=f"qr{m}") for m in range(8)]

        es_s4 = ExitStack()
        s4 = es_s4.enter_context(tc.tile_pool(name="s4", bufs=3))
        es_s4t = ExitStack()
        s4t = es_s4t.enter_context(tc.tile_pool(name="s4t", bufs=2))
        es_ps4 = ExitStack()
        ps4 = es_ps4.enter_context(tc.tile_pool(name="ps4", bufs=4, space="PSUM"))
        for cch in range(2):
            cs = slice(TC * cch, TC * (cch + 1))
            for mt in range(MT_QB):
                wt = s4.tile([128, KT_Q, 128], BF, tag="wqb", name="wqb")
                nc.sync.dma_start(out=wt, in_=wqb_in[:, mt])
                ps = ps4.tile([128, TC], F32, tag="ps", name="ps")
                for kt in range(KT_Q):
                    nc.tensor.matmul(ps, wt[:, kt, :], qn[kt][:, cs],
                                     start=(kt == 0), stop=(kt == KT_Q - 1))
                if mt < HC:
                    nc.scalar.copy(qh[mt][:, cs], ps)
                else:
                    rt = mt - HC
                    p_bf = s4t.tile([128, TC], BF, tag="p_bf", name="p_bf")
                    nc.scalar.copy(p_bf, ps)
                    p_sw = s4t.tile([128, TC], BF, tag="p_sw", name="p_sw")
                    nc.sync.dma_start(out=p_sw[0:32], in_=p_bf[32:64])
                    nc.sync.dma_start(out=p_sw[32:64], in_=p_bf[0:32])
                    nc.sync.dma_start(out=p_sw[64:96], in_=p_bf[96:128])
                    nc.sync.dma_start(out=p_sw[96:128], in_=p_bf[64:96])
                    t1 = s4t.tile([128, TC], BF, tag="t1", name="t1")
                    t2 = s4t.tile([128, TC], BF, tag="t2", name="t2")
                    nc.vector.tensor_mul(t1, p_bf, CCb[:, cs])
                    nc.vector.tensor_mul(t2, p_sw, SSb[:, cs])
                    nc.vector.tensor_add(qr[rt][:, cs], t1, t2)
        es_ps4.close()
        es_s4t.close()
        es_s4.close()
        es_qnkv.close()

        # ============ Attention (16 heads) ============
        es_oap = ExitStack()
        oap = es_oap.enter_context(tc.tile_pool(name="oap", bufs=1))
        o_all = [oap.tile([128, T], BF, tag=f"oa{m}", name=f"oa{m}")
                 for m in range(HC)]

        es_s6 = ExitStack()
        s6 = es_s6.enter_context(tc.tile_pool(name="s6", bufs=4))
        es_s6b = ExitStack()
        s6b = es_s6b.enter_context(tc.tile_pool(name="s6b", bufs=2))
        es_psS = ExitStack()
        psS = es_psS.enter_context(tc.tile_pool(name="psS", bufs=2, space="PSUM"))
        es_psO = ExitStack()
        psO = es_psO.enter_context(tc.tile_pool(name="psO", bufs=2, space="PSUM"))
        es_psD = ExitStack()
        psD = es_psD.enter_context(tc.tile_pool(name="psD", bufs=1, space="PSUM"))
        den4 = None
        for hh in range(HC):
            rt, half = hh // 2, hh % 2
            kp_t = kroped2a if half == 0 else kroped2b
            if hh % 4 == 0:
                den4 = s6b.tile([4, T], F32, tag="den4", name="den4")
            pso = [psO.tile([128, 512], F32, tag=f"o{c}", name=f"o{c}") for c in range(2)]
            psd = [psD.tile([1, 512], F32, tag=f"d{c}", name=f"d{c}") for c in range(2)]
            for jt in range(8):
                qlo = 128 * jt
                for ch in range(2):
                    ns, ne = max(qlo, 512 * ch), 512 * (ch + 1)
                    if ns >= ne:
                        continue
                    w = ne - ns
                    ost = ns - 512 * ch
                    jlast = 3 if ch == 0 else 7
                    pst = psS.tile([128, 512], F32, tag="s", name="s")
                    nc.tensor.matmul(pst[:, 0:w], kn[hh][:, qlo:qlo + 128],
                                     qh[hh][:, ns:ne], start=True, stop=False)
                    nc.tensor.matmul(pst[:, 0:w], kp_t[:, qlo:qlo + 128],
                                     qr[rt][:, ns:ne], start=False, stop=True)
                    et = s6.tile([128, 512], BF, tag="et", name="et")
                    nc.scalar.activation(out=et[:, 0:w], in_=pst[:, 0:w],
                                         func=mybir.ActivationFunctionType.Exp,
                                         scale=SCALE)
                    if ns == qlo:
                        nc.vector.tensor_mul(et[:, 0:128], et[:, 0:128], mask_t)
                    nc.tensor.matmul(pso[ch][:, ost:512],
                                     v_t[jt][:, 128 * hh:128 * (hh + 1)],
                                     et[:, 0:w], start=(jt == 0), stop=(jt == jlast),
                                     skip_group_check=True)
                    nc.tensor.matmul(psd[ch][:, ost:512], ones_bf, et[:, 0:w],
                                     start=(jt == 0), stop=(jt == jlast),
                                     skip_group_check=True)
            den_sb = s6.tile([1, T], F32, tag="den_sb", name="den_sb")
            for ch in range(2):
                cs = slice(512 * ch, 512 * (ch + 1))
                nc.scalar.copy(den_sb[0:1, cs], psd[ch])
                nc.scalar.copy(o_all[hh][:, cs], pso[ch])
            nc.sync.dma_start(out=den4[hh % 4:hh % 4 + 1], in_=den_sb)
            if hh % 4 == 3:
                g = hh // 4
                rec = s6b.tile([4, T], F32, tag="rec", name="rec")
                nc.vector.reciprocal(out=rec, in_=den4)
                recb = s6b.tile([4, T], BF, tag="recb", name="recb")
                nc.vector.tensor_copy(recb, rec)
                for h2 in range(4 * g, 4 * g + 4):
                    i4 = h2 - 4 * g
                    obf = s6b.tile([128, T], BF, tag="obf", name="obf")
                    for ch in range(2):
                        cs = slice(512 * ch, 512 * (ch + 1))
                        bcd = psS.tile([128, 512], F32, tag="s", name="s")
                        nc.tensor.matmul(bcd, e_all[:, 128 * i4:128 * (i4 + 1)],
                                         recb[:, cs], start=True, stop=True)
                        nc.vector.tensor_mul(obf[:, cs], o_all[h2][:, cs], bcd)
                    nc.sync.dma_start(
                        out=ago_in[g][128 * i4:128 * (i4 + 1)], in_=obf)
                    if DEBUG:
                        df = s6b.tile([128, T], F32, tag="dbg", name="dbg")
                        nc.scalar.copy(df, obf)
                        nc.sync.dma_start(
                            out=dbg_attn[128 * h2:128 * (h2 + 1)], in_=df)
                nc.gpsimd.collective_compute(
                    "AllGather", mybir.AluOpType.bypass, replica_groups=RG,
                    ins=[ago_in[g].opt()], outs=[ago_out[g].opt()])
        es_psD.close()
        es_psO.close()
        es_psS.close()
        es_s6b.close()
        es_s6.close()
        es_oap.close()

        es_heads.close()

        # ============ o_proj (row shard, K = 16384, M-split 4+3) ============
        es_s7w = ExitStack()
        s7w = es_s7w.enter_context(tc.tile_pool(name="s7w", bufs=6))
        es_s7r = ExitStack()
        s7r = es_s7r.enter_context(tc.tile_pool(name="s7r", bufs=6))
        es_s7o = ExitStack()
        s7o = es_s7o.enter_context(tc.tile_pool(name="s7o", bufs=4))
        es_ps7 = ExitStack()
        ps7 = es_ps7.enter_context(tc.tile_pool(name="ps7", bufs=1, space="PSUM"))
        for mg, mtts in ((0, (0, 1, 2, 3)), (1, (4, 5, 6))):
            nm = len(mtts)
            pso7 = [ps7.tile([128, 512], F32, tag=f"m{m}", name=f"m{m}")
                    for m in range(2 * nm)]
            for i in range(4):
                for c in range(NC):
                    for j in range(4):
                        ktg = 16 * c + 4 * i + j
                        wt = s7w.tile([128, nm, 128], BF, tag="wo", name="wo")
                        nc.sync.dma_start(
                            out=wt, in_=wo_in[:, ktg, mtts[0]:mtts[0] + nm])
                        rh = s7r.tile([128, T], BF, tag="rh", name="rh")
                        nc.sync.dma_start(
                            out=rh,
                            in_=ago_out[i][512 * c + 128 * j:512 * c + 128 * (j + 1)])
                        st = (i == 0 and c == 0 and j == 0)
                        sp = (i == 3 and c == NC - 1 and j == 3)
                        for m_i in range(nm):
                            for ch in range(2):
                                nc.tensor.matmul(
                                    pso7[2 * m_i + ch], wt[:, m_i, :],
                                    rh[:, 512 * ch:512 * (ch + 1)],
                                    start=st, stop=sp)
            for m_i in range(nm):
                mtt = mtts[m_i]
                for ch in range(2):
                    ot = s7o.tile([128, 512], F32, tag="ot", name="ot")
                    nc.scalar.copy(ot, pso7[2 * m_i + ch])
                    nc.sync.dma_start(
                        out=out_ap[128 * mtt:128 * (mtt + 1),
                                   512 * ch:512 * (ch + 1)],
                        in_=ot)
        es_ps7.close()
        es_s7o.close()
        es_s7r.close()
        es_s7w.close()

        es_persist.close()
        es_dram.close()

    nc.finalize()
    return nc


def _prep_inputs(hidden_states, cos, sin, wq_a, q_ln_w, wq_b, wkv_a, kv_ln_w,
                 wkv_b, wo):
    """Host-side sharding + layout. Returns in_maps (list of dicts per core)."""
    h2 = np.ascontiguousarray(hidden_states.reshape(S, D).T)      # [D, T]
    xh = np.ascontiguousarray(
        h2.reshape(KT_X, 128, T).transpose(1, 0, 2)).astype(bf16)  # [128,56,T]

    cosT = np.ascontiguousarray(cos.reshape(T, DR).T).astype(np.float32)
    sinT = np.ascontiguousarray(sin.reshape(T, DR).T).astype(np.float32)
    CCh = np.vstack([cosT, cosT])
    SSh = np.vstack([-sinT[:32], sinT[32:], -sinT[:32], sinT[32:]])
    ccss = np.ascontiguousarray(np.hstack([CCh, SSh])).astype(bf16)

    wq_b_eff = (wq_b * q_ln_w[None, :]).astype(np.float32)
    wkv_b_eff = (wkv_b * kv_ln_w[None, :]).astype(np.float32)
    wq_b_r = wq_b_eff.reshape(H, DQ, RQ)
    wkv_b_r = wkv_b_eff.reshape(H, DN + DV, RKV)
    woT = wo.T                                                    # [16384, D]

    def lhst_tiles(lhsT, kt, mt):
        # [K, M] -> [128, mt, kt, 128]
        K, M = lhsT.shape
        return np.ascontiguousarray(
            lhsT.reshape(kt, 128, mt, 128).transpose(1, 2, 0, 3)).astype(bf16)

    e_all = np.zeros((4, 512), dtype=bf16)
    for i in range(4):
        e_all[i, 128 * i:128 * (i + 1)] = 1.0
    cst = np.array([[EPS, 1.0 / RQ], [EPS, 1.0 / RKV]], dtype=np.float32)

    in_maps = []
    for c in range(NC):
        # rotate x k-tiles so this core's kpe K-shard (k-tiles 7c..7c+7 of D)
        # is in local positions 0..6
        rot = np.r_[7 * c:7 * (c + 1), 0:7 * c, 7 * (c + 1):KT_X]
        m = {"ccss": ccss, "eall": e_all, "cst": cst}
        m["x"] = np.ascontiguousarray(xh[:, rot, :])
        # stage A slice: 192 q rows + 64 kv rows  -> lhsT [D, 256], k-rotated
        qs = wq_a[192 * c:192 * (c + 1)]                           # [192, D]
        ks = wkv_a[64 * c:64 * (c + 1)]                            # [64, D]
        lhsT_a = np.vstack([qs, ks]).T                             # [D, 256]
        wa_t = lhst_tiles(lhsT_a, KT_X, 2).transpose(0, 2, 1, 3)   # [128,kt,2,128]
        m["wa"] = np.ascontiguousarray(wa_t[:, rot])
        # k_pe K-shard: wkv_a rows 512:576, K cols 896c..896(c+1)
        lhsT_kpe = wkv_a[RKV:RKV + DR, 896 * c:896 * (c + 1)].T    # [896, 64]
        m["wkpe"] = np.ascontiguousarray(
            lhsT_kpe.reshape(7, 128, DR).transpose(1, 0, 2)).astype(bf16)
        hs = slice(HC * c, HC * (c + 1))
        wq_b_c = wq_b_r[hs]                                        # [16,192,RQ]
        lhsT_qb = np.vstack([
            wq_b_c[:, :DN, :].reshape(HC * DN, RQ),
            wq_b_c[:, DN:, :].reshape(HC * DR, RQ)]).T             # [RQ, 3072]
        m["wqb"] = lhst_tiles(lhsT_qb, KT_Q, MT_QB)
        lhsT_kn = wkv_b_r[hs][:, :DN, :].reshape(HC * DN, RKV).T   # [RKV, 2048]
        m["wkn"] = lhst_tiles(lhsT_kn, KT_KV, HC)
        rhs_v = wkv_b_r[hs][:, DN:, :].reshape(HC * DV, RKV).T     # [RKV, 2048]
        m["wv"] = np.ascontiguousarray(
            rhs_v.reshape(KT_KV, 128, HC * DV).transpose(1, 0, 2)).astype(bf16)
        lhsT_wo = woT[:, 896 * c:896 * (c + 1)]                    # [16384, 896]
        m["wo"] = lhst_tiles(lhsT_wo, H * DV // 128, 7).transpose(0, 2, 1, 3).copy()
        in_maps.append(m)
    return in_maps


def _get_nc():
    if "nc" not in _CACHE:
        _CACHE["nc"] = _build()
    return _CACHE["nc"]


def run(in_maps, trace=False, trace_kwargs=None):
    nc = _get_nc()
    return run_bass_kernel_spmd(nc, in_maps, list(range(NC)), trace=trace,
                                **(trace_kwargs or {}))


def kernel(hidden_states, cos, sin, wq_a, q_ln_w, wq_b, wkv_a, kv_ln_w,
           wkv_b, wo):
    in_maps = _prep_inputs(hidden_states, cos, sin, wq_a, q_ln_w, wq_b,
                           wkv_a, kv_ln_w, wkv_b, wo)
    res = run(in_maps)
    out = np.concatenate([res.results[c]["out"] for c in range(NC)], axis=0)
    return np.ascontiguousarray(out.T).reshape(B, S, D).astype(np.float32)
